# revision 10
# baseline (speedup 1.0000x reference)
"""Trainium2 Bass kernel for nn_Attention_76647986365039.

Full multi-head attention returning (out, p_attn), matching the jax
reference:
    scores = q @ k^T / sqrt(d);  scores[mask==0] = -1e9
    p_attn = softmax(scores, axis=-1)
    out    = p_attn @ v

Distribution: batch*heads (64) sharded across 8 NeuronCores (8 heads each,
data/head parallel, no cross-core communication).

Per-core formulation (k-major "transposed scores"):
  - host pre-transposes q,k -> qT,kT [d, s] and the mask -> additive bias
    biasT[k,q] = (mask[q,k]-1)*32768 in fp8e5m2 (exact 0 / -32768 values).
  - scores^T chunk [128k x 512q] = kT_chunk.T @ qT  (fp32r matmul, d=64)
  - mask bias added by an fp8 identity matmul accumulating into the same
    PSUM bank (PE does the mask-add, not DVE).
  - one ScalarE pass: p_unnorm = Exp(0.125 * psum)  (PSUM -> SBUF, fp32r)
  - p_unnorm @ [v | 1] accumulated over the 16 k-chunks gives out^T (rows
    0..63) and the softmax row-sums (row 64) in one set of matmuls.
  - 1/sums = Exp(-Ln(sums)) on ScalarE after a rank-1 matmul broadcasts the
    sums row to all partitions; p is normalized on DVE in groups of CGRP
    k-chunks and DMA'd out k-major (1 MB per DMA); the host transposes
    p_attn/out back.
"""

import numpy as np
import ml_dtypes
from contextlib import ExitStack

import concourse.bass as bass
import concourse.tile as tile
import concourse.mybir as mybir
from concourse.bass_utils import run_bass_kernel_spmd
from concourse.masks import make_identity

B, H, S, D = 4, 16, 2048, 64
N_CORES = 8
HPC = (B * H) // N_CORES          # heads per core
QB = 512                          # q-block width (one PSUM bank)
NQB = S // QB                     # q-blocks per head
CHUNKS = S // 128                 # k-chunks per head
BIG = 32768.0                     # additive mask magnitude (fp8e5m2-exact)
SCALE = 1.0 / np.sqrt(D)          # 0.125
CGRP = 4                          # k-chunks per coalesced p tile / DMA
GP_NORM = ()                      # group indices normalized on GPSIMD (model: DVE-only wins)

F32 = mybir.dt.float32
F32R = mybir.dt.float32r
FP8 = mybir.dt.float8e5


def split_sync_waits(nc: bass.Bass, limit: int = 1):
    """This walrus build rejects instructions carrying more than one sync
    wait; move extra waits onto single-wait NOPs on the same engine."""
    for f in nc.m.functions:
        for blk in f.blocks:
            insts = blk.instructions
            out = []
            for inst in insts:
                si = inst.sync_info
                waits = list(si.on_wait) if si and si.on_wait else []
                if len(waits) > limit:
                    for w in waits[:-limit]:
                        nop = mybir.InstNoOp(
                            name=nc.get_next_instruction_name(), ins=[], outs=[]
                        )
                        nop.engine = inst.engine
                        nop.sync_info = mybir.SyncInfo(on_wait=[w], on_update=[])
                        out.append(nop)
                    inst.sync_info = mybir.SyncInfo(
                        on_wait=waits[-limit:], on_update=list(si.on_update or [])
                    )
                out.append(inst)
            insts[:] = out


def build_bass() -> bass.Bass:
    nc = bass.Bass("TRN2", debug=False, num_devices=N_CORES)
    qT = nc.dram_tensor("qT", [HPC, D, S], F32R, kind="ExternalInput")
    kT = nc.dram_tensor("kT", [HPC, D, S], F32R, kind="ExternalInput")
    v = nc.dram_tensor("v", [HPC, S, D + 1], F32R, kind="ExternalInput")
    biasT = nc.dram_tensor("biasT", [S, S], FP8, kind="ExternalInput")
    pT = nc.dram_tensor("pT", [HPC, S, S], F32, kind="ExternalOutput")
    outT = nc.dram_tensor("outT", [HPC, D, S], F32, kind="ExternalOutput")

    with tile.TileContext(nc) as tc, ExitStack() as ctx:
        singles = ctx.enter_context(tc.tile_pool(name="singles", bufs=1))
        heads = ctx.enter_context(tc.tile_pool(name="heads", bufs=2))
        punp = ctx.enter_context(tc.tile_pool(name="punp", bufs=6))
        pnp = ctx.enter_context(tc.tile_pool(name="pnp", bufs=6))
        outp = ctx.enter_context(tc.tile_pool(name="outp", bufs=2))
        smalls = ctx.enter_context(tc.tile_pool(name="smalls", bufs=3))
        spsum = ctx.enter_context(tc.tile_pool(name="spsum", bufs=4, space="PSUM"))
        opsum = ctx.enter_context(tc.tile_pool(name="opsum", bufs=2, space="PSUM"))
        bpsum = ctx.enter_context(tc.tile_pool(name="bpsum", bufs=2, space="PSUM"))

        ident = singles.tile([128, 128], FP8)
        make_identity(nc, ident)
        ones65 = singles.tile([65, 128], F32)
        nc.vector.memset(ones65, 1.0)

        bias_sb = singles.tile([128, CHUNKS, S], FP8)
        nc.sync.dma_start(bias_sb, biasT.rearrange("(c p) q -> p c q", p=128))

        for h in range(HPC):
            qT_sb = heads.tile([D, S], F32R, tag="qT")
            kT_sb = heads.tile([D, S], F32R, tag="kT")
            v1_sb = heads.tile([128, CHUNKS, D + 1], F32R, tag="v1")
            nc.sync.dma_start(qT_sb, qT[h])
            nc.sync.dma_start(kT_sb, kT[h])
            nc.sync.dma_start(
                v1_sb, v[h].rearrange("(c p) d -> p c d", p=128)
            )

            for qb in range(NQB):
                qsl = bass.ts(qb, QB)
                o_ps = opsum.tile([D + 1, QB], F32, tag="opsum")
                p_groups = []
                for c in range(CHUNKS):
                    if c % CGRP == 0:
                        p_grp = punp.tile([128, CGRP, QB], F32R, tag="pun")
                        p_groups.append(p_grp)
                    s_ps = spsum.tile([128, QB], F32, tag="s")
                    nc.tensor.matmul(
                        s_ps,
                        lhsT=kT_sb[:, bass.ts(c, 128)],
                        rhs=qT_sb[:, qsl],
                        start=True,
                        stop=False,
                    )
                    nc.tensor.matmul(
                        s_ps,
                        lhsT=ident,
                        rhs=bias_sb[:, c, qsl],
                        start=False,
                        stop=True,
                        skip_group_check=True,
                    )
                    p_un = p_grp[:, c % CGRP, :]
                    nc.scalar.activation(
                        p_un,
                        s_ps,
                        mybir.ActivationFunctionType.Exp,
                        scale=float(SCALE),
                    )
                    nc.tensor.matmul(
                        o_ps,
                        lhsT=v1_sb[:, c, :],
                        rhs=p_un,
                        start=(c == 0),
                        stop=(c == CHUNKS - 1),
                        skip_group_check=True,
                    )

                # sums live in o_ps row 64; replicate 1/sums to all partitions
                sums_sb = smalls.tile([65, QB], F32, tag="sums")
                nc.scalar.copy(sums_sb[64:65, :], o_ps[64:65, :])
                b_ps = bpsum.tile([128, QB], F32, tag="bcast")
                nc.tensor.matmul(
                    b_ps,
                    lhsT=ones65[64:65, :],
                    rhs=sums_sb[64:65, :],
                    start=True,
                    stop=True,
                )
                lnb = smalls.tile([128, QB], F32, tag="lnb")
                nc.scalar.activation(
                    lnb, b_ps, mybir.ActivationFunctionType.Ln, scale=1.0
                )
                recip = smalls.tile([128, QB], F32, tag="recip")
                nc.scalar.activation(
                    recip, lnb, mybir.ActivationFunctionType.Exp, scale=-1.0
                )

                o_sb = outp.tile([D, QB], F32, tag="o")
                nc.vector.tensor_tensor(
                    o_sb, o_ps[:D, :], recip[:D, :], mybir.AluOpType.mult
                )
                nc.sync.dma_start(outT[h, :, qsl], o_sb)

                pT_dst = pT[h].rearrange("(c p) q -> p c q", p=128)
                recip_b = recip[:, None, :].to_broadcast([128, CGRP, QB])
                for g in range(CHUNKS // CGRP):
                    eng = nc.gpsimd if g in GP_NORM else nc.vector
                    p_n = pnp.tile([128, CGRP, QB], F32, tag="pn")
                    eng.tensor_tensor(
                        p_n, p_groups[g], recip_b, mybir.AluOpType.mult
                    )
                    nc.sync.dma_start(
                        pT_dst[:, bass.ts(g, CGRP), qsl], p_n
                    )

    split_sync_waits(nc)
    return nc


_NC_CACHE = None


def _get_nc():
    global _NC_CACHE
    if _NC_CACHE is None:
        _NC_CACHE = build_bass()
    return _NC_CACHE


def kernel(q, k, v, mask):
    q = np.ascontiguousarray(np.asarray(q, dtype=np.float32))
    k = np.ascontiguousarray(np.asarray(k, dtype=np.float32))
    v = np.ascontiguousarray(np.asarray(v, dtype=np.float32))
    mask = np.asarray(mask)

    qf = q.reshape(B * H, S, D)
    kf = k.reshape(B * H, S, D)
    vf = v.reshape(B * H, S, D)
    v1f = np.concatenate(
        [vf, np.ones((B * H, S, 1), np.float32)], axis=-1
    )  # [BH, S, D+1]
    qT = np.ascontiguousarray(qf.transpose(0, 2, 1))  # [BH, D, S]
    kT = np.ascontiguousarray(kf.transpose(0, 2, 1))
    biasT = (
        (mask.reshape(S, S).T.astype(np.float32) - 1.0) * BIG
    ).astype(ml_dtypes.float8_e5m2)

    in_maps = []
    for i in range(N_CORES):
        sl = slice(i * HPC, (i + 1) * HPC)
        in_maps.append(
            {
                "qT": np.ascontiguousarray(qT[sl]),
                "kT": np.ascontiguousarray(kT[sl]),
                "v": np.ascontiguousarray(v1f[sl]),
                "biasT": biasT,
            }
        )

    nc = _get_nc()
    res = run_bass_kernel_spmd(nc, in_maps, core_ids=list(range(N_CORES)))

    pT_all = np.concatenate([r["pT"] for r in res.results], axis=0)  # [BH, S(k), S(q)]
    oT_all = np.concatenate([r["outT"] for r in res.results], axis=0)  # [BH, D, S]

    p_attn = np.ascontiguousarray(pT_all.transpose(0, 2, 1)).reshape(B, H, S, S)
    out = np.ascontiguousarray(oT_all.transpose(0, 2, 1)).reshape(B, H, S, D)
    return (out, p_attn)


# revision 11
# speedup vs baseline: 10.6287x; 10.6287x over previous
"""Trainium2 Bass kernel for nn_Attention_76647986365039.

Full multi-head attention returning (out, p_attn), matching the jax
reference:
    scores = q @ k^T / sqrt(d);  scores[mask==0] = -1e9
    p_attn = softmax(scores, axis=-1)
    out    = p_attn @ v

Distribution: batch*heads (64) sharded across 8 NeuronCores (8 heads each,
data/head parallel, no cross-core communication).

Per-core formulation (k-major "transposed scores"):
  - host pre-transposes q,k -> qT,kT [d, s] and the mask -> additive bias
    biasT[k,q] = (mask[q,k]-1)*32768 in fp8e5m2 (exact 0 / -32768 values).
  - scores^T chunk [128k x 512q] = kT_chunk.T @ qT  (fp32r matmul, d=64)
  - mask bias added by an fp8 identity matmul accumulating into the same
    PSUM bank (PE does the mask-add, not DVE).
  - one ScalarE pass: p_unnorm = Exp(0.125 * psum)  (PSUM -> SBUF, fp32r)
  - p_unnorm @ [v | 1] accumulated over the 16 k-chunks gives out^T (rows
    0..63) and the softmax row-sums (row 64) in one set of matmuls.
  - 1/sums = Exp(-Ln(sums)) on ScalarE after a rank-1 matmul broadcasts the
    sums row to all partitions; p is normalized on DVE in groups of CGRP
    k-chunks and DMA'd out k-major (1 MB per DMA); the host transposes
    p_attn/out back.
"""

import numpy as np
import ml_dtypes
from contextlib import ExitStack

import concourse.bass as bass
import concourse.tile as tile
import concourse.mybir as mybir
from concourse.bass_utils import run_bass_kernel_spmd
from concourse.masks import make_identity

B, H, S, D = 4, 16, 2048, 64
N_CORES = 8
HPC = (B * H) // N_CORES          # heads per core
QB = 512                          # q-block width (one PSUM bank)
NQB = S // QB                     # q-blocks per head
CHUNKS = S // 128                 # k-chunks per head
BIG = 32768.0                     # additive mask magnitude (fp8e5m2-exact)
SCALE = 1.0 / np.sqrt(D)          # 0.125
CGRP = 4                          # k-chunks per coalesced p tile / DMA
GP_NORM = ()                      # group indices normalized on GPSIMD (model: DVE-only wins)

F32 = mybir.dt.float32
F32R = mybir.dt.float32r
FP8 = mybir.dt.float8e5


def split_sync_waits(nc: bass.Bass, limit: int = 1):
    """This walrus build rejects instructions carrying more than one sync
    wait; move extra waits onto single-wait NOPs on the same engine."""
    for f in nc.m.functions:
        for blk in f.blocks:
            insts = blk.instructions
            out = []
            for inst in insts:
                si = inst.sync_info
                waits = list(si.on_wait) if si and si.on_wait else []
                if len(waits) > limit:
                    for w in waits[:-limit]:
                        nop = mybir.InstNoOp(
                            name=nc.get_next_instruction_name(), ins=[], outs=[]
                        )
                        nop.engine = inst.engine
                        nop.sync_info = mybir.SyncInfo(on_wait=[w], on_update=[])
                        out.append(nop)
                    inst.sync_info = mybir.SyncInfo(
                        on_wait=waits[-limit:], on_update=list(si.on_update or [])
                    )
                out.append(inst)
            insts[:] = out


def build_bass() -> bass.Bass:
    nc = bass.Bass("TRN2", debug=False, num_devices=N_CORES)
    qT = nc.dram_tensor("qT", [HPC, D, S], F32R, kind="ExternalInput")
    kT = nc.dram_tensor("kT", [HPC, D, S], F32R, kind="ExternalInput")
    v = nc.dram_tensor("v", [HPC, 128, CHUNKS, D + 1], F32R, kind="ExternalInput")
    biasT = nc.dram_tensor("biasT", [S, S], FP8, kind="ExternalInput")
    pT = nc.dram_tensor("pT", [HPC, S, S], F32, kind="ExternalOutput")
    outT = nc.dram_tensor("outT", [HPC, D, S], F32, kind="ExternalOutput")

    with tile.TileContext(nc) as tc, ExitStack() as ctx:
        singles = ctx.enter_context(tc.tile_pool(name="singles", bufs=1))
        heads = ctx.enter_context(tc.tile_pool(name="heads", bufs=2))
        punp = ctx.enter_context(tc.tile_pool(name="punp", bufs=6))
        pnp = ctx.enter_context(tc.tile_pool(name="pnp", bufs=6))
        outp = ctx.enter_context(tc.tile_pool(name="outp", bufs=2))
        smalls = ctx.enter_context(tc.tile_pool(name="smalls", bufs=3))
        spsum = ctx.enter_context(tc.tile_pool(name="spsum", bufs=4, space="PSUM"))
        opsum = ctx.enter_context(tc.tile_pool(name="opsum", bufs=2, space="PSUM"))
        bpsum = ctx.enter_context(tc.tile_pool(name="bpsum", bufs=2, space="PSUM"))

        ident = singles.tile([128, 128], FP8)
        make_identity(nc, ident)
        ones65 = singles.tile([65, 128], F32)
        nc.vector.memset(ones65, 1.0)

        bias_sb = singles.tile([128, CHUNKS, S], FP8)
        bias_src = biasT.rearrange("(c p) q -> p c q", p=128)

        for h in range(HPC):
            qT_sb = heads.tile([D, S], F32R, tag="qT")
            kT_sb = heads.tile([D, S], F32R, tag="kT")
            v1_sb = heads.tile([128, CHUNKS, D + 1], F32R, tag="v1")
            nc.sync.dma_start(qT_sb, qT[h])
            nc.sync.dma_start(kT_sb, kT[h])
            nc.sync.dma_start(v1_sb, v[h])
            if h == 0:
                # split the mask-bias load so chunk 0 compute starts early
                for bg in range(4):
                    bsl = bass.ts(bg, CHUNKS // 4)
                    nc.sync.dma_start(bias_sb[:, bsl, :], bias_src[:, bsl, :])

            for qb in range(NQB):
                qsl = bass.ts(qb, QB)
                o_ps = opsum.tile([D + 1, QB], F32, tag="opsum")
                p_groups = []
                for c in range(CHUNKS):
                    if c % CGRP == 0:
                        p_grp = punp.tile([128, CGRP, QB], F32R, tag="pun")
                        p_groups.append(p_grp)
                    s_ps = spsum.tile([128, QB], F32, tag="s")
                    nc.tensor.matmul(
                        s_ps,
                        lhsT=kT_sb[:, bass.ts(c, 128)],
                        rhs=qT_sb[:, qsl],
                        start=True,
                        stop=False,
                    )
                    nc.tensor.matmul(
                        s_ps,
                        lhsT=ident,
                        rhs=bias_sb[:, c, qsl],
                        start=False,
                        stop=True,
                        skip_group_check=True,
                    )
                    p_un = p_grp[:, c % CGRP, :]
                    nc.scalar.activation(
                        p_un,
                        s_ps,
                        mybir.ActivationFunctionType.Exp,
                        scale=float(SCALE),
                    )
                    nc.tensor.matmul(
                        o_ps,
                        lhsT=v1_sb[:, c, :],
                        rhs=p_un,
                        start=(c == 0),
                        stop=(c == CHUNKS - 1),
                        skip_group_check=True,
                    )

                # sums live in o_ps row 64; replicate 1/sums to all partitions
                sums_sb = smalls.tile([65, QB], F32, tag="sums")
                nc.scalar.copy(sums_sb[64:65, :], o_ps[64:65, :])
                b_ps = bpsum.tile([128, QB], F32, tag="bcast")
                nc.tensor.matmul(
                    b_ps,
                    lhsT=ones65[64:65, :],
                    rhs=sums_sb[64:65, :],
                    start=True,
                    stop=True,
                )
                lnb = smalls.tile([128, QB], F32, tag="lnb")
                nc.scalar.activation(
                    lnb, b_ps, mybir.ActivationFunctionType.Ln, scale=1.0
                )
                recip = smalls.tile([128, QB], F32, tag="recip")
                nc.scalar.activation(
                    recip, lnb, mybir.ActivationFunctionType.Exp, scale=-1.0
                )

                o_sb = outp.tile([D, QB], F32, tag="o")
                nc.vector.tensor_tensor(
                    o_sb, o_ps[:D, :], recip[:D, :], mybir.AluOpType.mult
                )
                nc.sync.dma_start(outT[h, :, qsl], o_sb)

                pT_dst = pT[h].rearrange("(c p) q -> p c q", p=128)
                recip_b = recip[:, None, :].to_broadcast([128, CGRP, QB])
                for g in range(CHUNKS // CGRP):
                    eng = nc.gpsimd if g in GP_NORM else nc.vector
                    p_n = pnp.tile([128, CGRP, QB], F32, tag="pn")
                    eng.tensor_tensor(
                        p_n, p_groups[g], recip_b, mybir.AluOpType.mult
                    )
                    nc.sync.dma_start(
                        pT_dst[:, bass.ts(g, CGRP), qsl], p_n
                    )

    split_sync_waits(nc)
    return nc


_NC_CACHE = None


def _get_nc():
    global _NC_CACHE
    if _NC_CACHE is None:
        _NC_CACHE = build_bass()
    return _NC_CACHE


def kernel(q, k, v, mask):
    q = np.ascontiguousarray(np.asarray(q, dtype=np.float32))
    k = np.ascontiguousarray(np.asarray(k, dtype=np.float32))
    v = np.ascontiguousarray(np.asarray(v, dtype=np.float32))
    mask = np.asarray(mask)

    qf = q.reshape(B * H, S, D)
    kf = k.reshape(B * H, S, D)
    vf = v.reshape(B * H, S, D)
    v1f = np.concatenate(
        [vf, np.ones((B * H, S, 1), np.float32)], axis=-1
    )  # [BH, S, D+1]
    # pre-pack v into the SBUF tile layout [128, CHUNKS, D+1] so the DMA
    # reads 4KB-contiguous runs per partition (sub-512B runs cost 2x)
    v1p = np.ascontiguousarray(
        v1f.reshape(B * H, CHUNKS, 128, D + 1).transpose(0, 2, 1, 3)
    )  # [BH, 128, CHUNKS, D+1]
    qT = np.ascontiguousarray(qf.transpose(0, 2, 1))  # [BH, D, S]
    kT = np.ascontiguousarray(kf.transpose(0, 2, 1))
    biasT = (
        (mask.reshape(S, S).T.astype(np.float32) - 1.0) * BIG
    ).astype(ml_dtypes.float8_e5m2)

    in_maps = []
    for i in range(N_CORES):
        sl = slice(i * HPC, (i + 1) * HPC)
        in_maps.append(
            {
                "qT": np.ascontiguousarray(qT[sl]),
                "kT": np.ascontiguousarray(kT[sl]),
                "v": np.ascontiguousarray(v1p[sl]),
                "biasT": biasT,
            }
        )

    nc = _get_nc()
    res = run_bass_kernel_spmd(nc, in_maps, core_ids=list(range(N_CORES)))

    pT_all = np.concatenate([r["pT"] for r in res.results], axis=0)  # [BH, S(k), S(q)]
    oT_all = np.concatenate([r["outT"] for r in res.results], axis=0)  # [BH, D, S]

    p_attn = np.ascontiguousarray(pT_all.transpose(0, 2, 1)).reshape(B, H, S, S)
    out = np.ascontiguousarray(oT_all.transpose(0, 2, 1)).reshape(B, H, S, D)
    return (out, p_attn)
